# revision 49
# baseline (speedup 1.0000x reference)
"""Trainium2 Bass kernel for nn_MultiHeadAttention_63591285785308 (v2).

Reference semantics (faithful "reshape, no transpose" head split):
  Q = query @ Wq.T + bq            [B, S, D]
  Qh = Q.reshape(B, H, S, dk)       # head h <-> rows h*128:(h+1)*128 of Q[b]
  scores = Qh @ Kh^T / sqrt(dk); P = softmax(scores); ctx = P @ Vh
  out = ctx.reshape(B, S, D) @ Wo.T + bo

Unit (b, h) depends only on the 128-row slab query[b, h*128:(h+1)*128] (same
for key/value) and writes only out[b, h*128:(h+1)*128]; 32 units are sharded
4-per-core.  Head positions use the j-major permutation s' = j*128 + r
(true position s = 16*r + j); attention is permutation-invariant along s.

v2 design (CoreSim-cost-model driven):
  - Q/K projected in "natural" orientation [out-dim block, seq] so the
    PSUM->SBUF copy quantizes straight into scores operands; bias folded
    into the PE accumulation as a rank-1 matmul (lhsT=bias row, rhs=ones).
  - bk dropped entirely: softmax_k(q . (k_j + b)) is invariant to the
    constant-in-j term, and the denominator divides the rest out exactly.
  - scores in fp8e4 DoubleRow with a stride-0 "k-group" dim (result is
    2*K^T Q, absorbed into the exp scale 0.0625): 0.5 cycles/row.
  - V projected flipped ([slab row, out-dim]), so PV streams V columns:
    ctx^T[q, d] accumulated over 16 key blocks at N=65/instr (64 d + a
    ones column that yields the softmax denominator for free).
  - ctx^T normalized (DVE) then PE-transposed back to [d, q] for the
    out-projection.
  - exp on ACT from [128, 1024] PSUM score tiles (ACT is the critical
    engine, ~133us); PE emission interleaves next-unit scores between
    PV passes so ACT never starves.
"""

import json

import numpy as np
import ml_dtypes

B, S, D, H, DK, P = 2, 2048, 1024, 16, 64, 128
NCORES = 8
UPC = 4  # units per core
HALF = 1024  # query positions per half-pass

_BF16 = ml_dtypes.bfloat16
_prog_cache = {}

_MAX_SYNC = 2  # this walrus build allows at most 2 sync commands per instruction


def _legalize_bir_sync(bir_bytes):
    """Split sync waits/updates exceeding the per-instruction cap onto
    adjacent same-engine NoOps (engine program order preserves semantics:
    waits move to preceding nops, update overflow to trailing nops)."""
    d = json.loads(bir_bytes)
    ctr = [0]

    def nop(engine, debug, waits, upds):
        ctr[0] += 1
        return {
            "debug": debug,
            "engine": engine,
            "ins": [],
            "name": f"I-lgl{ctr[0]}",
            "opcode": "NoOp",
            "outs": [],
            "sync_info": {"on_update": upds, "on_wait": waits},
        }

    changed = False
    for fn in d["functions"]:
        for blk in fn["blocks"]:
            new = []
            for ins in blk["instructions"]:
                si = ins.get("sync_info") or {}
                waits = list(si.get("on_wait") or [])
                upds = list(si.get("on_update") or [])
                if len(waits) + len(upds) <= _MAX_SYNC:
                    new.append(ins)
                    continue
                changed = True
                keep_u = upds[:_MAX_SYNC]
                extra_u = upds[_MAX_SYNC:]
                keep_w = waits[: max(0, _MAX_SYNC - len(keep_u))]
                extra_w = waits[len(keep_w):]
                # NoOp lowers to a CTRL-type op whose sync budget is 1 on
                # some engines (Pool) — put exactly one wait/update per nop.
                for w in extra_w:
                    new.append(nop(ins["engine"], ins.get("debug", 0), [w], []))
                si["on_wait"] = keep_w
                si["on_update"] = keep_u
                ins["sync_info"] = si
                new.append(ins)
                for uu in extra_u:
                    new.append(nop(ins["engine"], ins.get("debug", 0), [], [uu]))
            blk["instructions"] = new
    if not changed:
        return bir_bytes
    return json.dumps(d).encode()


def _install_bir_legalizer():
    if _prog_cache.get("legalizer_installed"):
        return
    from concourse import bass2jax

    orig = bass2jax.compile_bir_kernel

    def patched(ant_bir_str, compile_dir, neff_name="file.neff", **kw):
        return orig(_legalize_bir_sync(ant_bir_str), compile_dir, neff_name=neff_name, **kw)

    bass2jax.compile_bir_kernel = patched
    _prog_cache["legalizer_installed"] = True


def _build_program():
    import concourse.bass as bass
    import concourse.mybir as mybir
    import concourse.tile as tile
    from concourse.vector_clock import ScopedClock, VectorClock
    from concourse.masks import make_identity

    dt = mybir.dt
    BF = dt.bfloat16
    F32 = dt.float32
    F8 = dt.float8e4
    MUL = mybir.AluOpType.mult
    EXP = mybir.ActivationFunctionType.Exp
    DR = mybir.MatmulPerfMode.DoubleRow

    class SplitDrainTileContext(tile.TileContext):
        """This walrus build caps sem waits per instruction below what the
        stock tail drain needs; split the waits across single-wait SP nops
        (SP program order then gates the bare drain)."""

        def _drain_and_barrier(self, tick_clock, wait_clock):
            gc = tick_clock.global_clock
            for proc in range(len(gc)):
                tick = gc[proc]
                if tick <= 0:
                    continue
                vc = VectorClock()
                vc.require_at_least(proc, tick)
                nop = self.nc.sync.nop(nofuse=True)
                wait_clock.add_sem_waits(nop.ins, ScopedClock({None: vc}))
            self.nc.sync.drain()
            self.nc.all_engine_barrier()
            assert self.sems is not None
            popped = self.nc._tile_sem_poison_stack.pop()
            assert popped is self._sem_poison
            self.nc.clear_and_free_semaphores(list(self.sems.allocated().values()))
            self.nc.all_engine_barrier()

    nc = bass.Bass()

    xq_d = nc.declare_dram_parameter("xqT", [D, 512], BF, isOutput=False)
    xk_d = nc.declare_dram_parameter("xkT", [D, 512], BF, isOutput=False)
    xv_d = nc.declare_dram_parameter("xvT", [D, 512], BF, isOutput=False)
    wq_d = nc.declare_dram_parameter("wqT", [D, D], BF, isOutput=False)
    wk_d = nc.declare_dram_parameter("wkT", [D, D], BF, isOutput=False)
    wv_d = nc.declare_dram_parameter("wvT", [D, D], BF, isOutput=False)
    wo_d = nc.declare_dram_parameter("woT", [D, D], BF, isOutput=False)
    bq_d = nc.declare_dram_parameter("bq", [P, 8], F32, isOutput=False)
    bv_d = nc.declare_dram_parameter("bv", [1, D], BF, isOutput=False)
    bo_d = nc.declare_dram_parameter("bo", [1, D], BF, isOutput=False)
    out_d = nc.declare_dram_parameter("out", [UPC, P, D], F32, isOutput=True)

    with SplitDrainTileContext(nc) as tc:
        with (
            tc.tile_pool(name="persist", bufs=1) as pp,
            tc.tile_pool(name="pt", bufs=40) as ptp,
            tc.tile_pool(name="ctxn", bufs=2) as cnp,
            tc.tile_pool(name="gs", bufs=2) as gsp,
            tc.tile_pool(name="ost", bufs=4) as osp,
            tc.tile_pool(name="rec", bufs=4) as rcp,
            tc.tile_pool(name="sc_ps", bufs=2, space="PSUM") as scp,
            tc.tile_pool(name="ctx_ps", bufs=2, space="PSUM") as cxp,
            tc.tile_pool(name="g_ps", bufs=1, space="PSUM") as gpp,
            tc.tile_pool(name="o_ps", bufs=1, space="PSUM") as opp,
        ):
            # ---- persistent SBUF tiles
            w_sb = {nm: pp.tile([P, 8, D], BF, name=f"w_{nm}", tag=f"w_{nm}")
                    for nm in ("q", "k", "v", "o")}
            x_sb = {nm: pp.tile([P, 8, 512], BF, name=f"x_{nm}", tag=f"x_{nm}")
                    for nm in ("q", "k", "v")}
            bq_sb = pp.tile([P, 8], F32, name="bq", tag="bq")
            bv_sb = pp.tile([1, D], BF, name="bv", tag="bv")
            bo_sb = pp.tile([1, D], BF, name="bo", tag="bo")
            q8 = [pp.tile([P, 1, S], F8, name=f"q8_{pr}", tag=f"q8_{pr}") for pr in range(2)]
            k8 = [pp.tile([P, 1, S], F8, name=f"k8_{pr}", tag=f"k8_{pr}") for pr in range(2)]
            vsl = [pp.tile([P, 16, 65], BF, name=f"vsl_{u}", tag=f"vsl_{u}") for u in range(UPC)]

            # ---- DMA emission, startup-critical first
            wqr = wq_d.rearrange("(c p) o -> p c o", p=P)
            wkr = wk_d.rearrange("(c p) o -> p c o", p=P)
            wvr = wv_d.rearrange("(c p) o -> p c o", p=P)
            wor = wo_d.rearrange("(c p) o -> p c o", p=P)
            xqr = xq_d.rearrange("(c p) s -> p c s", p=P)
            xkr = xk_d.rearrange("(c p) s -> p c s", p=P)
            xvr = xv_d.rearrange("(c p) s -> p c s", p=P)
            nc.sync.dma_start(out=x_sb["q"][:], in_=xqr[:])
            nc.sync.dma_start(out=w_sb["q"][:, :, 0:256], in_=wqr[:, :, 0:256])
            nc.sync.dma_start(out=bq_sb[:], in_=bq_d[:])
            nc.sync.dma_start(out=x_sb["k"][:], in_=xkr[:])
            nc.sync.dma_start(out=w_sb["k"][:, :, 0:256], in_=wkr[:, :, 0:256])
            nc.sync.dma_start(out=w_sb["q"][:, :, 256:512], in_=wqr[:, :, 256:512])
            nc.sync.dma_start(out=w_sb["k"][:, :, 256:512], in_=wkr[:, :, 256:512])
            nc.sync.dma_start(out=w_sb["q"][:, :, 512:D], in_=wqr[:, :, 512:D])
            nc.sync.dma_start(out=w_sb["k"][:, :, 512:D], in_=wkr[:, :, 512:D])
            nc.sync.dma_start(out=x_sb["v"][:], in_=xvr[:])
            for i in range(2):
                nc.sync.dma_start(out=w_sb["v"][:, i * 4:(i + 1) * 4, :],
                                  in_=wvr[:, i * 4:(i + 1) * 4, :])
            nc.sync.dma_start(out=bv_sb[:], in_=bv_d[:])
            for i in range(2):
                nc.sync.dma_start(out=w_sb["o"][:, i * 4:(i + 1) * 4, :],
                                  in_=wor[:, i * 4:(i + 1) * 4, :])
            nc.sync.dma_start(out=bo_sb[:], in_=bo_d[:])

            ident = pp.tile([P, P], BF, name="ident", tag="ident")
            make_identity(nc, ident)
            ones_row = pp.tile([1, 512], BF, name="ones_row", tag="ones_row")
            nc.gpsimd.memset(ones_row, 1.0)
            ones16 = pp.tile([P, 16], BF, name="ones16", tag="ones16")
            nc.gpsimd.memset(ones16, 1.0)
            for u in range(UPC):
                nc.vector.tensor_copy(out=vsl[u][:, :, 64], in_=ones16[:])
            # dummy exp: pulls the ACT Exp table load off the critical path
            tblw = pp.tile([1, 16], F32, name="tblw", tag="tblw")
            nc.scalar.activation(tblw[:], ones16[0:1, :], EXP)

            # PE warm-up: the tensor engine's clock ramps over its first
            # ~3us of sustained use; burn the ramp on dummy transposes
            # while the first DMAs are still in flight.
            warm = gpp.tile([P, 8, P], BF, tag="g", name="warm")
            for _ in range(110):
                nc.tensor.matmul(
                    warm[0:P, 0, 0:P], lhsT=ident[:], rhs=ident[:],
                    is_transpose=True, start=True, stop=True,
                )

            # ---- helpers (emission only; Tile framework handles sync)
            def proj_qk(nm, c, pr):
                # one device-pair (2 units, 256 x-columns) at a time, so the
                # first scores need only a quarter of the x/w DMA.  Proj psum
                # stays OFF the scores pool (gpp/opp/cxp are idle then).
                if nm == "q":
                    t = cxp.tile([P, 4, P], F32, tag="cx", name=f"pj_{nm}{c}_{pr}")
                    pjp3 = t[:, 0:2, 0:P]
                    pjp = pjp3
                else:
                    pool = gpp if c % 2 == 0 else opp
                    t = pool.tile([P, 512], F32, tag="g" if c % 2 == 0 else "o",
                                  name=f"pj_{nm}{c}_{pr}")
                    pjp = t[:, 0:256]
                for i in range(8):
                    nc.tensor.matmul(
                        pjp,
                        lhsT=w_sb[nm][:, i, c * P:(c + 1) * P],
                        rhs=x_sb[nm][:, i, pr * 256:(pr + 1) * 256],
                        start=(i == 0),
                        stop=(i == 7),
                    )
                dst = q8 if nm == "q" else k8
                for hh in range(2):
                    j = 2 * c + hh
                    for pu in range(2):
                        out_sl = dst[pr][pu * 64:pu * 64 + 64, 0, j * P:(j + 1) * P]
                        if nm == "q":
                            in_sl = pjp3[hh * 64:(hh + 1) * 64, pu, :]
                            # fold the Q bias into the quantizing copy
                            nc.vector.tensor_scalar_add(
                                out=out_sl, in0=in_sl,
                                scalar1=bq_sb[hh * 64:(hh + 1) * 64, c:c + 1],
                            )
                        else:
                            in_sl = pjp[hh * 64:(hh + 1) * 64, pu * P:(pu + 1) * P]
                            nc.vector.tensor_copy(out=out_sl, in_=in_sl)

            def proj_v(u):
                for ot in range(2):
                    t = cxp.tile([P, 4, P], F32, tag="cx", name=f"pv{u}_{ot}")
                    pjp = t[:, 0:4, 0:P]
                    for i in range(8):
                        nc.tensor.matmul(
                            pjp,
                            lhsT=x_sb["v"][:, i, u * P:(u + 1) * P],
                            rhs=w_sb["v"][:, i, ot * 512:(ot + 1) * 512],
                            start=(i == 0), stop=False,
                        )
                    nc.tensor.matmul(
                        pjp,
                        lhsT=ones_row[0:1, 0:P],
                        rhs=bv_sb[0:1, ot * 512:(ot + 1) * 512],
                        start=False, stop=True,
                    )
                    nc.vector.tensor_copy(out=vsl[u][:, ot * 8:(ot + 1) * 8, 0:64], in_=pjp)

            def scores_kbs(u, h, kbs, segs):
                pr, pu = u // 2, u % 2
                prow = slice(pu * 64, pu * 64 + 64)
                for kb in kbs:
                    t = scp.tile([P, 1024], F32, tag="ps", name=f"sc{u}_{h}_{kb}")
                    lhsT = k8[pr][prow, 0:1, kb * P:(kb + 1) * P].to_broadcast((64, 2, P))
                    for i in range(2):
                        q0 = h * HALF + i * 512
                        rhs = q8[pr][prow, 0:1, q0:q0 + 512].to_broadcast((64, 2, 512))
                        nc.tensor.matmul(
                            t[:, i * 512:(i + 1) * 512],
                            lhsT=lhsT, rhs=rhs,
                            start=True, stop=True,
                            perf_mode=DR,
                        )
                    pt = ptp.tile([P, HALF], BF, tag="pt", name=f"pt{u}_{h}_{kb}")
                    nc.scalar.activation(pt[:], t[:], EXP, scale=0.0625)
                    segs.setdefault(kb, []).append((pt, 0, 8))

            def scores_kb_i(u, h, kb, i, segs):
                # half-width (512-col) score tile: one DR matmul + one exp.
                pr, pu = u // 2, u % 2
                prow = slice(pu * 64, pu * 64 + 64)
                t = scp.tile([P, 1024], F32, tag="ps", name=f"sch{u}_{h}_{kb}_{i}")
                lhsT = k8[pr][prow, 0:1, kb * P:(kb + 1) * P].to_broadcast((64, 2, P))
                q0 = h * HALF + i * 512
                rhs = q8[pr][prow, 0:1, q0:q0 + 512].to_broadcast((64, 2, 512))
                nc.tensor.matmul(
                    t[:, 0:512], lhsT=lhsT, rhs=rhs,
                    start=True, stop=True, perf_mode=DR,
                )
                pt = ptp.tile([P, HALF], BF, tag="pt", name=f"pth{u}_{h}_{kb}_{i}")
                nc.scalar.activation(pt[:, 0:512], t[:, 0:512], EXP, scale=0.0625)
                segs.setdefault(kb, []).append((pt, i * 4, 4))

            def pv_half(u, h, pts, ctxn_u, alt_pool=False):
                """PV + normalize for one q-half.  alt_pool rotates the ctx
                psum bank through the (by-then idle) scores pool as well, so
                all four qg groups overlap (used on the final half only)."""
                cxs = []
                for qg in range(2):
                    pool = scp if (alt_pool and qg == 1) else cxp
                    cxs.append(pool.tile([P, 4, P], F32,
                                         tag="cx" if pool is cxp else "ps",
                                         name=f"cx{u}_{h}_{qg}"))
                # kb-major so only the final 8 matmuls depend on the last
                # exp tile of the half; one accumulation group per bank
                # (start zeroes the whole 2KB zero region).
                for kb in range(16):
                    for qb in range(8):
                        for (ptile, qb0, nqb) in pts[kb]:
                            if qb0 <= qb < qb0 + nqb:
                                lhsT = ptile[:, (qb - qb0) * P:(qb - qb0 + 1) * P]
                                break
                        nc.tensor.matmul(
                            cxs[qb // 4][:, qb % 4, 0:65],
                            lhsT=lhsT,
                            rhs=vsl[u][:, kb, :],
                            start=(qb % 4 == 0 and kb == 0),
                            stop=(qb % 4 == 3 and kb == 15),
                        )
                for qg in range(2):
                    cx = cxs[qg]
                    rec = rcp.tile([P, 4, 1], F32, tag="rec", name=f"rec{u}_{h}_{qg}")
                    nc.vector.reciprocal(rec[:, :, 0], cx[:, :, 64])
                    j0 = h * 8 + qg * 4
                    nc.vector.tensor_tensor(
                        out=ctxn_u[:, j0:j0 + 4, :],
                        in0=cx[:, :, 0:64],
                        in1=rec[:, :, 0:1].to_broadcast((P, 4, DK)),
                        op=MUL,
                    )

            fin_state = {}

            def finish_part(u, ctxn_u, crange, par_ot=False):
                """Transposes + out-projection for the given D-chunk range.
                The last call (containing c=7) adds bias, stages, DMAs."""
                if u not in fin_state:
                    gp = gpp.tile([P, 8, P], BF, tag="g", name=f"gp{u}")
                    gs = gsp.tile([P, 8, P], BF, tag="gs", name=f"gs{u}")
                    ops_l = []
                    for ot in range(2):
                        if ot == 1 and par_ot:
                            # run ot1 out of a (by-then idle) ctx bank so it
                            # doesn't serialize behind ot0's staging copy
                            opt = cxp.tile([P, 4, P], F32, tag="cx", name=f"op{u}_{ot}")
                            ops_l.append(opt[:, 0:4, 0:P])
                        else:
                            opt = opp.tile([P, 512], F32, tag="o", name=f"op{u}_{ot}")
                            ops_l.append(opt[:])
                    fin_state[u] = (gp, gs, ops_l)
                gp, gs, ops_l = fin_state[u]
                c0, c1 = crange[0], crange[-1]
                for c in crange:
                    for hh in range(2):
                        j = 2 * c + hh
                        nc.tensor.matmul(
                            gp[hh * 64:(hh + 1) * 64, c, :],
                            lhsT=ctxn_u[:, j, :],
                            rhs=ident[:],
                            is_transpose=True,
                            start=True, stop=True,
                        )
                nc.vector.tensor_copy(out=gs[:, c0:c1 + 1, :], in_=gp[:, c0:c1 + 1, :])
                for ot in range(2):
                    ops = ops_l[ot]
                    for c in crange:
                        nc.tensor.matmul(
                            ops,
                            lhsT=gs[:, c, :],
                            rhs=w_sb["o"][:, c, ot * 512:(ot + 1) * 512],
                            start=(c == 0), stop=False,
                        )
                    if c1 == 7:
                        nc.tensor.matmul(
                            ops,
                            lhsT=ones_row[0:1, 0:P],
                            rhs=bo_sb[0:1, ot * 512:(ot + 1) * 512],
                            start=False, stop=True,
                        )
                        og = osp.tile([P, 512], F32, tag="og", name=f"og{u}_{ot}")
                        nc.vector.tensor_copy(out=og[:], in_=ops)
                        nc.sync.dma_start(out=out_d[u, :, ot * 512:(ot + 1) * 512], in_=og[:])

            def finish_unit(u, ctxn_u, par_ot=False):
                finish_part(u, ctxn_u, range(8), par_ot=par_ot)

            # ---- emission schedule
            pts = {(u, h): {} for u in range(UPC) for h in range(2)}
            ctxn = {}

            # fine-grained start: half-width score tiles as soon as the
            # first two Q c-blocks and K c-block 0 are projected.  Units 0
            # and 1 share the pair's q8/k8 tiles, so both units' h0 scores
            # interleave through the whole prologue — that doubles the ACT
            # exp feed and matches the K/Q projection cadence.
            proj_qk("q", 0)
            proj_qk("q", 1)
            proj_qk("k", 0)
            for kb in (0, 1):
                for uu in (0, 1):
                    scores_kb_i(uu, 0, kb, 0, pts[(uu, 0)])
            proj_qk("k", 1)
            for kb in (2, 3):
                for uu in (0, 1):
                    scores_kb_i(uu, 0, kb, 0, pts[(uu, 0)])
            proj_qk("k", 2)
            for kb in (4, 5):
                for uu in (0, 1):
                    scores_kb_i(uu, 0, kb, 0, pts[(uu, 0)])
            proj_qk("q", 2)
            proj_qk("q", 3)
            for kb in range(6):
                for uu in (0, 1):
                    scores_kb_i(uu, 0, kb, 1, pts[(uu, 0)])
            for c in range(3, 8):
                proj_qk("k", c)
                scores_kbs(0, 0, [2 * c, 2 * c + 1], pts[(0, 0)])
                scores_kbs(1, 0, [2 * c, 2 * c + 1], pts[(1, 0)])
            for c in range(4, 8):
                proj_qk("q", c)
            scores_kbs(0, 1, range(16), pts[(0, 1)])

            def pv0(u, alt=False):
                ctxn[u] = cnp.tile([P, 16, DK], BF, tag="cn", name=f"cn{u}")
                pv_half(u, 0, pts[(u, 0)], ctxn[u], alt_pool=alt)

            proj_v(0)
            pv0(0)
            scores_kbs(1, 1, range(16), pts[(1, 1)])
            proj_v(1)
            pv_half(0, 1, pts[(0, 1)], ctxn[0])
            scores_kbs(2, 0, range(16), pts[(2, 0)])
            pv0(1)
            scores_kbs(2, 1, range(16), pts[(2, 1)])
            pv_half(1, 1, pts[(1, 1)], ctxn[1])
            scores_kbs(3, 0, range(16), pts[(3, 0)])
            finish_unit(0, ctxn[0])
            proj_v(2)
            pv0(2)
            scores_kbs(3, 1, range(16), pts[(3, 1)])
            finish_unit(1, ctxn[1])
            pv_half(2, 1, pts[(2, 1)], ctxn[2])
            finish_unit(2, ctxn[2])
            proj_v(3)
            pv0(3)
            finish_part(3, ctxn[3], range(0, 4), par_ot=True)
            pv_half(3, 1, pts[(3, 1)], ctxn[3], alt_pool=True)
            finish_part(3, ctxn[3], range(4, 8), par_ot=True)

    return nc


def _get_program():
    if "nc" not in _prog_cache:
        _prog_cache["nc"] = _build_program()
    return _prog_cache["nc"]


def _prepare_in_maps(query, key, value, Wq, bq, Wk, bk, Wv, bv, Wo, bo):
    wqT = np.ascontiguousarray(Wq.T).astype(_BF16)
    wkT = np.ascontiguousarray(Wk.T).astype(_BF16)
    wvT = np.ascontiguousarray(Wv.T).astype(_BF16)
    woT = np.ascontiguousarray(Wo.T).astype(_BF16)
    bq2 = np.ascontiguousarray(np.asarray(bq).reshape(8, P).T).astype(np.float32)
    bv2 = np.asarray(bv).reshape(1, D).astype(_BF16)
    bo2 = np.asarray(bo).reshape(1, D).astype(_BF16)

    in_maps = []
    for core in range(NCORES):
        units = [core * UPC + k for k in range(UPC)]
        slabs = {}
        for nm, full in (("xqT", query), ("xkT", key), ("xvT", value)):
            cols = [
                np.ascontiguousarray(full[u // H, (u % H) * P:(u % H + 1) * P, :].T)
                for u in units
            ]
            slabs[nm] = np.concatenate(cols, axis=1).astype(_BF16)
        in_maps.append(
            {
                **slabs,
                "wqT": wqT, "wkT": wkT, "wvT": wvT, "woT": woT,
                "bq": bq2, "bv": bv2, "bo": bo2,
            }
        )
    return in_maps


def kernel(query, key, value, Wq, bq, Wk, bk, Wv, bv, Wo, bo, _trace=False):
    from concourse.bass_utils import run_bass_kernel_spmd

    _install_bir_legalizer()

    query = np.asarray(query, dtype=np.float32)
    key = np.asarray(key, dtype=np.float32)
    value = np.asarray(value, dtype=np.float32)

    nc = _get_program()
    in_maps = _prepare_in_maps(query, key, value,
                               np.asarray(Wq), np.asarray(bq), np.asarray(Wk),
                               np.asarray(bk), np.asarray(Wv), np.asarray(bv),
                               np.asarray(Wo), np.asarray(bo))
    core_ids = list(range(NCORES))
    res = run_bass_kernel_spmd(nc, in_maps, core_ids, trace=_trace)
    _prog_cache["last_results"] = res

    out = np.empty((B, S, D), np.float32)
    for core in range(NCORES):
        o = res.results[core]["out"]
        for k in range(UPC):
            u = core * UPC + k
            out[u // H, (u % H) * P:(u % H + 1) * P, :] = o[k]
    return out
